# revision 29
# baseline (speedup 1.0000x reference)
"""Bahdanau (additive) attention kernel for Trainium2, 8 NeuronCores.

Math (per batch b):
    q = query @ W1                        (t, u)
    k = value @ W2                        (s, u)
    scores[t, s] = sum_u scale_u * tanh(q[t, u] + k[s, u])
    scores = where(mask[s], scores, -1e9)
    attn = softmax_s(scores)
    context = attn @ value

Sharding: data-parallel over batch — 16 batches, 2 per core, W1/W2/scale
replicated. Each core runs an identical Bass program (SPMD).

Per-core device algorithm (measured ~289 us/core, engine-balance limited:
ScalarE tanh stream vs VectorE broadcast-adds — the 268M
elementwise tanh stream is the fundamental cost and tanh exists only on
ScalarE at 1 elem/lane/cycle):
  - Prep: qT (u, t) fp32 and kT (u, s) fp16 built with PE transposes +
    fp16 matmuls so the contraction dim `u` lies on partitions. fp16 is used
    on every matmul path (fp32 matmuls lower to 2x LOW_HIGH passes); fp16
    keeps attn rel err ~5e-4.
  - Main loop over (32-row t-group, u-tile j), batches interleaved per group
    so batch boundaries pipeline: the tanh argument arg[u_j, s] =
    kT[u_j, s] + qT[u_j, t] is a broadcast-add; all 32 rows/group go through DVE
    tensor_scalar (per-partition scalar = qT column, ~280 ns/row — DVE
    per-instruction floor) into an fp16 staging tile consumed by one big
    ScalarE tanh (~230 ns/row); the other 5 rows/group fuse the add into
    ScalarE tanh (~230 ns/row). The per-partition-bias fusion path
    (~510 ns/row on ScalarE) lost to all-DVE staging once balance was judged
    by busy-interval UNION per engine rather than duration sums (instruction
    durations embed sem-waits and overstate DVE load ~30%).
  - Reduction over u on the TensorE: scale is replicated to a (128, 32)
    stationary operand (M=32) and each matmul streams two tanh rows
    (N=512, one PSUM bank); tile_position=(0, 32c) packs 4 such matvecs
    into disjoint column strips of the PE array, which both runs them
    concurrently and lands score rows on 4 different PSUM partitions
    (32c), so one full-width DVE copy evacuates 16 rows at once.
    PSUM bank rule honored: each matmul covers exactly one bank, so
    start=(j==0)/stop=(j==3) per slot.
  - Engines are partition-locked (no cross-partition moves), so score rows
    bounce through a DRAM staging buffer: 4 contiguous rows per strip-store,
    then one gather DMA rebuilds the (t, s) tile per half-batch.
  - softmax over the free dim without max-subtraction (|scores| <= 22 since
    |tanh|<=1 and sum|scale| ~ 22 -> exp stays comfortably in fp32 range);
    tails run per 64-row half-batch so they overlap the main loop.
  - context = attn @ value via PE transposes of attn + 2 matmuls.

Note: an all-masked row would produce NaN (reference's max-subtraction gives
uniform weights instead); the problem spec fixes mask = all-ones, and any
partially-masked row matches the reference exactly.
"""

import numpy as np
from contextlib import ExitStack

import concourse.bass as bass
from concourse import bacc
import concourse.tile as tile
from concourse import mybir
from concourse.bass import ts
from concourse.bass_utils import run_bass_kernel_spmd
from concourse.masks import make_identity

AF = mybir.ActivationFunctionType
F32 = mybir.dt.float32
F16 = mybir.dt.float16
U8 = mybir.dt.uint8

B, T, S, D, U = 16, 128, 256, 512, 512
NCORES = 8
BPC = B // NCORES  # batches per core
NJ = U // 128      # u-tiles
ND = D // 128      # d-tiles
NK = S // 128      # s-tiles
GA = 32            # t-rows per tanh group (2 PSUM sub-groups of 16)
DVE_ROWS = 32      # rows per group whose adds run on DVE (rest: ACT bias)
NGA = T // GA
NEG = -1e9


def build_bass():
    nc = bacc.Bacc("TRN2", target_bir_lowering=False, debug=False)

    query_d = nc.dram_tensor("query", [BPC, T, D], F32, kind="ExternalInput")
    value_d = nc.dram_tensor("value", [BPC, S, D], F32, kind="ExternalInput")
    mask_d = nc.dram_tensor("mask", [1, BPC, S], U8, kind="ExternalInput")
    w1_d = nc.dram_tensor("W1", [128, ND, U], F32, kind="ExternalInput")   # [p,i,u] = W1[i*128+p, u]
    w2_d = nc.dram_tensor("W2", [128, ND, U], F32, kind="ExternalInput")
    scale_d = nc.dram_tensor("scale", [128, NJ, 32], F16, kind="ExternalInput")  # [p,j,m] = scale[j*128+p]

    ctx_d = nc.dram_tensor("context", [BPC, T, D], F32, kind="ExternalOutput")
    attn_d = nc.dram_tensor("attn", [BPC, T, S], F32, kind="ExternalOutput")
    stage_d = nc.dram_tensor("scores_stage", [BPC, T * S], F32)  # internal DRAM bounce

    with tile.TileContext(nc) as tc, ExitStack() as ctx:
        consts = ctx.enter_context(tc.tile_pool(name="consts", bufs=1))
        pb = ctx.enter_context(tc.tile_pool(name="perbatch", bufs=2))
        stag_pool = ctx.enter_context(tc.tile_pool(name="stag", bufs=4))
        tanh_pool = ctx.enter_context(tc.tile_pool(name="tanh", bufs=4))
        row_pool = ctx.enter_context(tc.tile_pool(name="rowbuf", bufs=2))
        ps_small = ctx.enter_context(tc.tile_pool(name="ps_small", bufs=2, space="PSUM"))
        ps_rows = ctx.enter_context(tc.tile_pool(name="ps_rows", bufs=2, space="PSUM"))
        ps_ctx = ctx.enter_context(tc.tile_pool(name="ps_ctx", bufs=1, space="PSUM"))

        # ---- constants ----
        # Preload the tanh table set (~2.7us) while input DMAs run.
        warm = consts.tile([128, 1], F32)
        nc.vector.memset(warm[:], 0.0)
        nc.scalar.activation(warm[:], warm[:], AF.Tanh)

        ident = consts.tile([128, 128], F32)
        make_identity(nc, ident)

        scale16 = consts.tile([128, NJ, 32], F16)
        nc.sync.dma_start(scale16[:], scale_d[:])
        # gpsimd DMA casts fp32 DRAM -> fp16 SBUF directly (keeps the big
        # weight loads off the sync queue so query/value DMAs go first)
        w1_16 = consts.tile([128, ND, U], F16)
        nc.gpsimd.dma_start(w1_16[:], w1_d[:])
        w2_16 = consts.tile([128, ND, U], F16)
        nc.gpsimd.dma_start(w2_16[:], w2_d[:])

        qTs, kTs, v_nats, mb_bcs = [], [], [], []

        def emit_prep(b):
            # ---- load ----
            q_nat = pb.tile([128, D], F32)                      # (t, d)
            nc.sync.dma_start(q_nat[:], query_d[b])
            v_nat = pb.tile([128, NK, D], F32)                  # (s%128, k, d)
            nc.sync.dma_start(v_nat[:], value_d[b].rearrange("(k p) d -> p k d", p=128))

            # mask bias broadcast to (128, S):  (mask-1)*1e9
            mb_u8 = pb.tile([128, S], U8)
            mask_bc = bass.AP(
                tensor=mask_d.ap().tensor, offset=b * S,
                ap=[[0, 128], [1, S]],
            )
            nc.sync.dma_start(mb_u8[:], mask_bc)
            mb_bc = pb.tile([128, S], F32)
            nc.vector.tensor_scalar(
                mb_bc[:], mb_u8[:], 1e9, NEG,
                mybir.AluOpType.mult, mybir.AluOpType.add,
            )

            # ---- transpose query -> qTin (d on partitions) ----
            qTin = pb.tile([128, ND, 128], F16)                 # (d%128, i, t)
            for i in range(ND):
                ps_t = ps_small.tile([128, 128], F32, tag="ps_prep")
                nc.tensor.transpose(ps_t[:], q_nat[:, ts(i, 128)], ident[:])
                nc.vector.tensor_copy(qTin[:, i, :], ps_t[:])

            # ---- transpose value -> vT (d on partitions) ----
            vT = pb.tile([128, ND, S], F16)                     # (d%128, i, s)
            for i in range(ND):
                for k in range(NK):
                    ps_t = ps_small.tile([128, 128], F32, tag="ps_prep")
                    nc.tensor.transpose(ps_t[:], v_nat[:, k, ts(i, 128)], ident[:])
                    nc.vector.tensor_copy(vT[:, i, ts(k, 128)], ps_t[:])

            # ---- qT[u_j, t] = sum_i W1[d_i, u_j].T @ qTin[d_i, t] ----
            qT = pb.tile([128, NJ, 128], F32)   # fp32: DVE scalar operand
            for j in range(NJ):
                ps_q = ps_small.tile([128, 128], F32, tag="ps_prep")
                for i in range(ND):
                    nc.tensor.matmul(
                        ps_q[:], w1_16[:, i, ts(j, 128)], qTin[:, i, :],
                        start=(i == 0), stop=(i == ND - 1),
                    )
                nc.vector.tensor_copy(qT[:, j, :], ps_q[:])

            # ---- kT[u_j, s] = sum_i W2[d_i, u_j].T @ vT[d_i, s] ----
            kT = pb.tile([128, NJ, S], F16)                     # (u%128, j, s)
            for j in range(NJ):
                ps_k = ps_small.tile([128, S], F32, tag="ps_prep")
                for i in range(ND):
                    nc.tensor.matmul(
                        ps_k[:], w2_16[:, i, ts(j, 128)], vT[:, i, :],
                        start=(i == 0), stop=(i == ND - 1),
                    )
                nc.vector.tensor_copy(kT[:, j, :], ps_k[:])
            qTs.append(qT); kTs.append(kT); v_nats.append(v_nat); mb_bcs.append(mb_bc)

        # ---- main loops, batches interleaved per row-group ----
        # Per GA=16-row group: DVE builds tanh args for the first DVE_ROWS[j]
        # rows via tensor_scalar broadcast-add; ScalarE handles the remaining
        # rows fused into its tanh via the per-partition bias operand.
        # Row pair p (rows 2p, 2p+1) -> PE col-strip c=p//2 (tile_position
        # (0,32c), M=32 replicated scale so a whole strip fills), PSUM half
        # h=p%2. Strip c holds rows [4c, 4c+4): one wide DVE copy evacuates
        # 16 rows, one DMA per strip stores 4 contiguous rows to a DRAM
        # staging buffer (engines cannot scatter across partitions).
        def emit_group(ga, b):
            if True:
                qT, kT = qTs[b], kTs[b]
                tanh_tiles = []
                for j in range(NJ):
                    stag = stag_pool.tile([128, DVE_ROWS * S], F16)
                    for r in range(DVE_ROWS):
                        t = ga * GA + r
                        nc.vector.tensor_scalar_add(
                            stag[:, ts(r, S)], kT[:, j, :], qT[:, j, t:t + 1],
                        )
                    tanh_t = tanh_pool.tile([128, GA * S], F16)
                    nc.scalar.activation(
                        tanh_t[:, 0:DVE_ROWS * S], stag[:], AF.Tanh)
                    for r in range(DVE_ROWS, GA):
                        t = ga * GA + r
                        nc.scalar.activation(
                            tanh_t[:, ts(r, S)], kT[:, j, :], AF.Tanh,
                            bias=qT[:, j, t:t + 1],
                        )
                    tanh_tiles.append(tanh_t)
                for sub in range(GA // 16):
                    prow = ps_rows.tile([128, 4 * S], F32)
                    for j in range(NJ):
                        for p in range(8):
                            c, h = p // 2, p % 2
                            r = sub * 16 + 2 * p
                            nc.tensor.matmul(
                                prow[32 * c:32 * c + 32, ts(h, 2 * S)],
                                scale16[:, j, :], tanh_tiles[j][:, r * S:(r + 2) * S],
                                start=(j == 0), stop=(j == NJ - 1),
                                tile_position=(0, 32 * c),
                                skip_group_check=True,
                            )
                    rowbuf = row_pool.tile([128, 4 * S], F32)
                    nc.vector.tensor_copy(rowbuf[:], prow[:])
                    for c in range(4):
                        base = (ga * GA + sub * 16 + 4 * c) * S
                        nc.sync.dma_start(
                            stage_d[b, base:base + 4 * S].rearrange("(o x) -> o x", o=1),
                            rowbuf[32 * c:32 * c + 1, :],
                        )

        # Emission order: batch-0 prep, then its first group immediately (so
        # ScalarE gets work ~20us earlier instead of idling behind both
        # batches' prep), then batch-1 prep, then the rest interleaved.
        emit_prep(0)
        emit_group(0, 0)
        emit_prep(1)
        for _ga in range(NGA):
            for _b in range(BPC):
                if (_ga, _b) != (0, 0):
                    emit_group(_ga, _b)

        # ---- tails: softmax + context, in half-batches so they overlap ----
        attnTs = {}
        ps_cs = {}
        for b in range(BPC):
            attnTs[b] = pb.tile([128, NK, 128], F32, name=f"attnT{b}", tag=f"attnT{b}")  # (s%128, k, t)
            ps_cs[b] = ps_ctx.tile([128, D], F32, name=f"ps_c{b}", tag=f"ps_c{b}")
        for b in range(BPC):
            for half in range(2):
                t0 = half * 64
                # gather staged scores rows [t0, t0+64)
                sc_h = pb.tile([64, S], F32, tag="sc_h")
                nc.sync.dma_start(
                    sc_h[:], stage_d[b, t0 * S:(t0 + 64) * S].rearrange("(t s) -> t s", s=S))
                masked = pb.tile([64, S], F32, tag="masked_h")
                nc.vector.tensor_add(masked[:], sc_h[:], mb_bcs[b][0:64, :])
                attn_e = pb.tile([64, S], F32, tag="attn_e_h")
                nc.scalar.activation(attn_e[:], masked[:], AF.Exp)
                ssum = pb.tile([64, 1], F32, tag="ssum_h")
                nc.vector.tensor_reduce(ssum[:], attn_e[:], axis=mybir.AxisListType.X,
                                        op=mybir.AluOpType.add)
                rsum = pb.tile([64, 1], F32, tag="rsum_h")
                nc.vector.reciprocal(rsum[:], ssum[:])
                attn_o = pb.tile([64, S], F32, tag="attn_o_h")
                nc.vector.tensor_scalar_mul(attn_o[:], attn_e[:], rsum[:])
                nc.sync.dma_start(attn_d[b, t0:t0 + 64, :], attn_o[:])

                # transpose this half into the batch attnT tile
                for k in range(NK):
                    ps_t = ps_small.tile([128, 64], F32, tag="ps_prep")
                    nc.tensor.transpose(ps_t[:], attn_o[:, ts(k, 128)], ident[0:64, 0:64])
                    nc.scalar.copy(attnTs[b][:, k, t0:t0 + 64], ps_t[:])
                # context rows [t0, t0+64): lhsT M=64 -> psum partition base t0
                for k in range(NK):
                    nc.tensor.matmul(
                        ps_cs[b][t0:t0 + 64, :], attnTs[b][:, k, t0:t0 + 64],
                        v_nats[b][:, k, :],
                        start=(k == 0), stop=(k == NK - 1),
                        skip_group_check=True,
                    )
            ctx_sb = pb.tile([128, D], F32)
            nc.scalar.copy(ctx_sb[:], ps_cs[b][:])
            nc.sync.dma_start(ctx_d[b], ctx_sb[:])

    nc.compile()
    return nc


_NC_CACHE = None


def _get_nc():
    global _NC_CACHE
    if _NC_CACHE is None:
        _NC_CACHE = build_bass()
    return _NC_CACHE


def _shard_inputs(query, value, mask, W1, W2, scale):
    w1_r = np.ascontiguousarray(
        np.asarray(W1, dtype=np.float32).reshape(ND, 128, U).transpose(1, 0, 2))
    w2_r = np.ascontiguousarray(
        np.asarray(W2, dtype=np.float32).reshape(ND, 128, U).transpose(1, 0, 2))
    scale_r = np.ascontiguousarray(np.broadcast_to(
        np.asarray(scale, dtype=np.float32).reshape(NJ, 128).T.astype(np.float16)[:, :, None],
        (128, NJ, 32)))
    in_maps = []
    for c in range(NCORES):
        sl = slice(c * BPC, (c + 1) * BPC)
        in_maps.append({
            "query": np.ascontiguousarray(np.asarray(query[sl], dtype=np.float32)),
            "value": np.ascontiguousarray(np.asarray(value[sl], dtype=np.float32)),
            "mask": np.ascontiguousarray(
                np.asarray(mask[sl]).astype(np.uint8).reshape(1, BPC, S)),
            "W1": w1_r,
            "W2": w2_r,
            "scale": scale_r,
        })
    return in_maps


def run(query, value, mask, W1, W2, scale, **run_kwargs):
    nc = _get_nc()
    in_maps = _shard_inputs(query, value, mask, W1, W2, scale)
    res = run_bass_kernel_spmd(nc, in_maps, core_ids=list(range(NCORES)), **run_kwargs)
    context = np.concatenate([r["context"] for r in res.results], axis=0)
    attn = np.concatenate([r["attn"] for r in res.results], axis=0)
    return (context, attn), res


def kernel(query, value, mask, W1, W2, scale):
    (context, attn), _ = run(query, value, mask, W1, W2, scale)
    return context, attn


# revision 30
# speedup vs baseline: 1.0067x; 1.0067x over previous
"""Bahdanau (additive) attention kernel for Trainium2, 8 NeuronCores.

Math (per batch b):
    q = query @ W1                        (t, u)
    k = value @ W2                        (s, u)
    scores[t, s] = sum_u scale_u * tanh(q[t, u] + k[s, u])
    scores = where(mask[s], scores, -1e9)
    attn = softmax_s(scores)
    context = attn @ value

Sharding: data-parallel over batch — 16 batches, 2 per core, W1/W2/scale
replicated. Each core runs an identical Bass program (SPMD).

Per-core device algorithm (measured ~289 us/core, engine-balance limited:
ScalarE tanh stream vs VectorE broadcast-adds — the 268M
elementwise tanh stream is the fundamental cost and tanh exists only on
ScalarE at 1 elem/lane/cycle):
  - Prep: qT (u, t) fp32 and kT (u, s) fp16 built with PE transposes +
    fp16 matmuls so the contraction dim `u` lies on partitions. fp16 is used
    on every matmul path (fp32 matmuls lower to 2x LOW_HIGH passes); fp16
    keeps attn rel err ~5e-4.
  - Main loop over (32-row t-group, u-tile j), batches interleaved per group
    so batch boundaries pipeline: the tanh argument arg[u_j, s] =
    kT[u_j, s] + qT[u_j, t] is a broadcast-add; all 32 rows/group go through DVE
    tensor_scalar (per-partition scalar = qT column, ~280 ns/row — DVE
    per-instruction floor) into an fp16 staging tile consumed by one big
    ScalarE tanh (~230 ns/row); the other 5 rows/group fuse the add into
    ScalarE tanh (~230 ns/row). The per-partition-bias fusion path
    (~510 ns/row on ScalarE) lost to all-DVE staging once balance was judged
    by busy-interval UNION per engine rather than duration sums (instruction
    durations embed sem-waits and overstate DVE load ~30%).
  - Reduction over u on the TensorE: scale is replicated to a (128, 32)
    stationary operand (M=32) and each matmul streams two tanh rows
    (N=512, one PSUM bank); tile_position=(0, 32c) packs 4 such matvecs
    into disjoint column strips of the PE array, which both runs them
    concurrently and lands score rows on 4 different PSUM partitions
    (32c), so one full-width DVE copy evacuates 16 rows at once.
    PSUM bank rule honored: each matmul covers exactly one bank, so
    start=(j==0)/stop=(j==3) per slot.
  - Engines are partition-locked (no cross-partition moves), so score rows
    bounce through a DRAM staging buffer: 4 contiguous rows per strip-store,
    then one gather DMA rebuilds the (t, s) tile per half-batch.
  - softmax over the free dim without max-subtraction (|scores| <= 22 since
    |tanh|<=1 and sum|scale| ~ 22 -> exp stays comfortably in fp32 range);
    tails run per 64-row half-batch so they overlap the main loop.
  - context = attn @ value via PE transposes of attn + 2 matmuls.

Note: an all-masked row would produce NaN (reference's max-subtraction gives
uniform weights instead); the problem spec fixes mask = all-ones, and any
partially-masked row matches the reference exactly.
"""

import numpy as np
from contextlib import ExitStack

import concourse.bass as bass
from concourse import bacc
import concourse.tile as tile
from concourse import mybir
from concourse.bass import ts
from concourse.bass_utils import run_bass_kernel_spmd
from concourse.masks import make_identity

AF = mybir.ActivationFunctionType
F32 = mybir.dt.float32
F16 = mybir.dt.float16
U8 = mybir.dt.uint8

B, T, S, D, U = 16, 128, 256, 512, 512
NCORES = 8
BPC = B // NCORES  # batches per core
NJ = U // 128      # u-tiles
ND = D // 128      # d-tiles
NK = S // 128      # s-tiles
GA = 32            # t-rows per tanh group (2 PSUM sub-groups of 16)
DVE_ROWS = 32      # rows per group whose adds run on DVE (rest: ACT bias)
NGA = T // GA
NEG = -1e9


def build_bass():
    nc = bacc.Bacc("TRN2", target_bir_lowering=False, debug=False)

    query_d = nc.dram_tensor("query", [BPC, T, D], F32, kind="ExternalInput")
    value_d = nc.dram_tensor("value", [BPC, S, D], F32, kind="ExternalInput")
    mask_d = nc.dram_tensor("mask", [1, BPC, S], U8, kind="ExternalInput")
    w1_d = nc.dram_tensor("W1", [128, ND, U], F32, kind="ExternalInput")   # [p,i,u] = W1[i*128+p, u]
    w2_d = nc.dram_tensor("W2", [128, ND, U], F32, kind="ExternalInput")
    scale_d = nc.dram_tensor("scale", [128, NJ, 32], F16, kind="ExternalInput")  # [p,j,m] = scale[j*128+p]

    ctx_d = nc.dram_tensor("context", [BPC, T, D], F32, kind="ExternalOutput")
    attn_d = nc.dram_tensor("attn", [BPC, T, S], F32, kind="ExternalOutput")
    stage_d = nc.dram_tensor("scores_stage", [BPC, T * S], F32)  # internal DRAM bounce

    with tile.TileContext(nc) as tc, ExitStack() as ctx:
        consts = ctx.enter_context(tc.tile_pool(name="consts", bufs=1))
        pb = ctx.enter_context(tc.tile_pool(name="perbatch", bufs=2))
        stag_pool = ctx.enter_context(tc.tile_pool(name="stag", bufs=4))
        tanh_pool = ctx.enter_context(tc.tile_pool(name="tanh", bufs=4))
        row_pool = ctx.enter_context(tc.tile_pool(name="rowbuf", bufs=2))
        ps_small = ctx.enter_context(tc.tile_pool(name="ps_small", bufs=2, space="PSUM"))
        ps_rows = ctx.enter_context(tc.tile_pool(name="ps_rows", bufs=2, space="PSUM"))
        ps_ctx = ctx.enter_context(tc.tile_pool(name="ps_ctx", bufs=1, space="PSUM"))

        # ---- constants ----
        # Preload the tanh table set (~2.7us) while input DMAs run.
        warm = consts.tile([128, 1], F32)
        nc.vector.memset(warm[:], 0.0)
        nc.scalar.activation(warm[:], warm[:], AF.Tanh)

        ident = consts.tile([128, 128], F32)
        make_identity(nc, ident)

        scale16 = consts.tile([128, NJ, 32], F16)
        nc.sync.dma_start(scale16[:], scale_d[:])
        # gpsimd DMA casts fp32 DRAM -> fp16 SBUF directly (keeps the big
        # weight loads off the sync queue so query/value DMAs go first)
        w1_16 = consts.tile([128, ND, U], F16)
        nc.gpsimd.dma_start(w1_16[:], w1_d[:])
        w2_16 = consts.tile([128, ND, U], F16)
        nc.gpsimd.dma_start(w2_16[:], w2_d[:])

        qTs, kTs, v_nats, mb_bcs = [], [], [], []
        for b in range(BPC):
            # ---- load ----
            q_nat = pb.tile([128, D], F32)                      # (t, d)
            nc.sync.dma_start(q_nat[:], query_d[b])
            v_nat = pb.tile([128, NK, D], F32)                  # (s%128, k, d)
            nc.sync.dma_start(v_nat[:], value_d[b].rearrange("(k p) d -> p k d", p=128))

            # mask bias broadcast to (128, S):  (mask-1)*1e9
            mb_u8 = pb.tile([128, S], U8)
            mask_bc = bass.AP(
                tensor=mask_d.ap().tensor, offset=b * S,
                ap=[[0, 128], [1, S]],
            )
            nc.sync.dma_start(mb_u8[:], mask_bc)
            mb_bc = pb.tile([128, S], F32)
            nc.vector.tensor_scalar(
                mb_bc[:], mb_u8[:], 1e9, NEG,
                mybir.AluOpType.mult, mybir.AluOpType.add,
            )

            # ---- transpose query -> qTin (d on partitions) ----
            qTin = pb.tile([128, ND, 128], F16)                 # (d%128, i, t)
            for i in range(ND):
                ps_t = ps_small.tile([128, 128], F32, tag="ps_prep")
                nc.tensor.transpose(ps_t[:], q_nat[:, ts(i, 128)], ident[:])
                nc.vector.tensor_copy(qTin[:, i, :], ps_t[:])

            # ---- transpose value -> vT (d on partitions) ----
            vT = pb.tile([128, ND, S], F16)                     # (d%128, i, s)
            for i in range(ND):
                for k in range(NK):
                    ps_t = ps_small.tile([128, 128], F32, tag="ps_prep")
                    nc.tensor.transpose(ps_t[:], v_nat[:, k, ts(i, 128)], ident[:])
                    nc.vector.tensor_copy(vT[:, i, ts(k, 128)], ps_t[:])

            # ---- qT[u_j, t] = sum_i W1[d_i, u_j].T @ qTin[d_i, t] ----
            qT = pb.tile([128, NJ, 128], F32)   # fp32: DVE scalar operand
            for j in range(NJ):
                ps_q = ps_small.tile([128, 128], F32, tag="ps_prep")
                for i in range(ND):
                    nc.tensor.matmul(
                        ps_q[:], w1_16[:, i, ts(j, 128)], qTin[:, i, :],
                        start=(i == 0), stop=(i == ND - 1),
                    )
                nc.vector.tensor_copy(qT[:, j, :], ps_q[:])

            # ---- kT[u_j, s] = sum_i W2[d_i, u_j].T @ vT[d_i, s] ----
            kT = pb.tile([128, NJ, S], F16)                     # (u%128, j, s)
            for j in range(NJ):
                ps_k = ps_small.tile([128, S], F32, tag="ps_prep")
                for i in range(ND):
                    nc.tensor.matmul(
                        ps_k[:], w2_16[:, i, ts(j, 128)], vT[:, i, :],
                        start=(i == 0), stop=(i == ND - 1),
                    )
                nc.vector.tensor_copy(kT[:, j, :], ps_k[:])
            qTs.append(qT); kTs.append(kT); v_nats.append(v_nat); mb_bcs.append(mb_bc)

        # ---- main loops, batches interleaved per row-group ----
        # Per GA=16-row group: DVE builds tanh args for the first DVE_ROWS[j]
        # rows via tensor_scalar broadcast-add; ScalarE handles the remaining
        # rows fused into its tanh via the per-partition bias operand.
        # Row pair p (rows 2p, 2p+1) -> PE col-strip c=p//2 (tile_position
        # (0,32c), M=32 replicated scale so a whole strip fills), PSUM half
        # h=p%2. Strip c holds rows [4c, 4c+4): one wide DVE copy evacuates
        # 16 rows, one DMA per strip stores 4 contiguous rows to a DRAM
        # staging buffer (engines cannot scatter across partitions).
        for ga in range(NGA):
            for b in range(BPC):
                qT, kT = qTs[b], kTs[b]
                tanh_tiles = []
                for j in range(NJ):
                    stag = stag_pool.tile([128, DVE_ROWS * S], F16)
                    for r in range(DVE_ROWS):
                        t = ga * GA + r
                        nc.vector.tensor_scalar_add(
                            stag[:, ts(r, S)], kT[:, j, :], qT[:, j, t:t + 1],
                        )
                    tanh_t = tanh_pool.tile([128, GA * S], F16)
                    nc.scalar.activation(
                        tanh_t[:, 0:DVE_ROWS * S], stag[:], AF.Tanh)
                    for r in range(DVE_ROWS, GA):
                        t = ga * GA + r
                        nc.scalar.activation(
                            tanh_t[:, ts(r, S)], kT[:, j, :], AF.Tanh,
                            bias=qT[:, j, t:t + 1],
                        )
                    tanh_tiles.append(tanh_t)
                for sub in range(GA // 16):
                    prow = ps_rows.tile([128, 4 * S], F32)
                    for j in range(NJ):
                        for p in range(8):
                            c, h = p // 2, p % 2
                            r = sub * 16 + 2 * p
                            nc.tensor.matmul(
                                prow[32 * c:32 * c + 32, ts(h, 2 * S)],
                                scale16[:, j, :], tanh_tiles[j][:, r * S:(r + 2) * S],
                                start=(j == 0), stop=(j == NJ - 1),
                                tile_position=(0, 32 * c),
                                skip_group_check=True,
                            )
                    rowbuf = row_pool.tile([128, 4 * S], F32)
                    nc.vector.tensor_copy(rowbuf[:], prow[:])
                    for c in range(4):
                        base = (ga * GA + sub * 16 + 4 * c) * S
                        nc.sync.dma_start(
                            stage_d[b, base:base + 4 * S].rearrange("(o x) -> o x", o=1),
                            rowbuf[32 * c:32 * c + 1, :],
                        )

        # ---- tails: softmax + context, in half-batches so they overlap ----
        attnTs = {}
        ps_cs = {}
        for b in range(BPC):
            attnTs[b] = pb.tile([128, NK, 128], F32, name=f"attnT{b}", tag=f"attnT{b}")  # (s%128, k, t)
            ps_cs[b] = ps_ctx.tile([128, D], F32, name=f"ps_c{b}", tag=f"ps_c{b}")
        for b in range(BPC):
            for half in range(2):
                t0 = half * 64
                # gather staged scores rows [t0, t0+64)
                sc_h = pb.tile([64, S], F32, tag="sc_h")
                nc.sync.dma_start(
                    sc_h[:], stage_d[b, t0 * S:(t0 + 64) * S].rearrange("(t s) -> t s", s=S))
                masked = pb.tile([64, S], F32, tag="masked_h")
                nc.vector.tensor_add(masked[:], sc_h[:], mb_bcs[b][0:64, :])
                attn_e = pb.tile([64, S], F32, tag="attn_e_h")
                nc.scalar.activation(attn_e[:], masked[:], AF.Exp)
                ssum = pb.tile([64, 1], F32, tag="ssum_h")
                nc.vector.tensor_reduce(ssum[:], attn_e[:], axis=mybir.AxisListType.X,
                                        op=mybir.AluOpType.add)
                rsum = pb.tile([64, 1], F32, tag="rsum_h")
                nc.vector.reciprocal(rsum[:], ssum[:])
                attn_o = pb.tile([64, S], F32, tag="attn_o_h")
                nc.vector.tensor_scalar_mul(attn_o[:], attn_e[:], rsum[:])
                nc.sync.dma_start(attn_d[b, t0:t0 + 64, :], attn_o[:])

                # transpose this half into the batch attnT tile
                for k in range(NK):
                    ps_t = ps_small.tile([128, 64], F32, tag="ps_prep")
                    nc.tensor.transpose(ps_t[:], attn_o[:, ts(k, 128)], ident[0:64, 0:64])
                    nc.scalar.copy(attnTs[b][:, k, t0:t0 + 64], ps_t[:])
                # context rows [t0, t0+64): lhsT M=64 -> psum partition base t0
                for k in range(NK):
                    nc.tensor.matmul(
                        ps_cs[b][t0:t0 + 64, :], attnTs[b][:, k, t0:t0 + 64],
                        v_nats[b][:, k, :],
                        start=(k == 0), stop=(k == NK - 1),
                        skip_group_check=True,
                    )
            ctx_sb = pb.tile([128, D], F32)
            nc.scalar.copy(ctx_sb[:], ps_cs[b][:])
            nc.sync.dma_start(ctx_d[b], ctx_sb[:])

    nc.compile()
    return nc


_NC_CACHE = None


def _get_nc():
    global _NC_CACHE
    if _NC_CACHE is None:
        _NC_CACHE = build_bass()
    return _NC_CACHE


def _shard_inputs(query, value, mask, W1, W2, scale):
    w1_r = np.ascontiguousarray(
        np.asarray(W1, dtype=np.float32).reshape(ND, 128, U).transpose(1, 0, 2))
    w2_r = np.ascontiguousarray(
        np.asarray(W2, dtype=np.float32).reshape(ND, 128, U).transpose(1, 0, 2))
    scale_r = np.ascontiguousarray(np.broadcast_to(
        np.asarray(scale, dtype=np.float32).reshape(NJ, 128).T.astype(np.float16)[:, :, None],
        (128, NJ, 32)))
    in_maps = []
    for c in range(NCORES):
        sl = slice(c * BPC, (c + 1) * BPC)
        in_maps.append({
            "query": np.ascontiguousarray(np.asarray(query[sl], dtype=np.float32)),
            "value": np.ascontiguousarray(np.asarray(value[sl], dtype=np.float32)),
            "mask": np.ascontiguousarray(
                np.asarray(mask[sl]).astype(np.uint8).reshape(1, BPC, S)),
            "W1": w1_r,
            "W2": w2_r,
            "scale": scale_r,
        })
    return in_maps


def run(query, value, mask, W1, W2, scale, **run_kwargs):
    nc = _get_nc()
    in_maps = _shard_inputs(query, value, mask, W1, W2, scale)
    res = run_bass_kernel_spmd(nc, in_maps, core_ids=list(range(NCORES)), **run_kwargs)
    context = np.concatenate([r["context"] for r in res.results], axis=0)
    attn = np.concatenate([r["attn"] for r in res.results], axis=0)
    return (context, attn), res


def kernel(query, value, mask, W1, W2, scale):
    (context, attn), _ = run(query, value, mask, W1, W2, scale)
    return context, attn


# revision 31
# speedup vs baseline: 1.0080x; 1.0012x over previous
"""Bahdanau (additive) attention kernel for Trainium2, 8 NeuronCores.

Math (per batch b):
    q = query @ W1                        (t, u)
    k = value @ W2                        (s, u)
    scores[t, s] = sum_u scale_u * tanh(q[t, u] + k[s, u])
    scores = where(mask[s], scores, -1e9)
    attn = softmax_s(scores)
    context = attn @ value

Sharding: data-parallel over batch — 16 batches, 2 per core, W1/W2/scale
replicated. Each core runs an identical Bass program (SPMD).

Per-core device algorithm (measured ~289 us/core, engine-balance limited:
ScalarE tanh stream vs VectorE broadcast-adds — the 268M
elementwise tanh stream is the fundamental cost and tanh exists only on
ScalarE at 1 elem/lane/cycle):
  - Prep: qT (u, t) fp32 and kT (u, s) fp16 built with PE transposes +
    fp16 matmuls so the contraction dim `u` lies on partitions. fp16 is used
    on every matmul path (fp32 matmuls lower to 2x LOW_HIGH passes); fp16
    keeps attn rel err ~5e-4.
  - Main loop over (32-row t-group, u-tile j), batches interleaved per group
    so batch boundaries pipeline: the tanh argument arg[u_j, s] =
    kT[u_j, s] + qT[u_j, t] is a broadcast-add; all 32 rows/group go through DVE
    tensor_scalar (per-partition scalar = qT column, ~280 ns/row — DVE
    per-instruction floor) into an fp16 staging tile consumed by one big
    ScalarE tanh (~230 ns/row); the other 5 rows/group fuse the add into
    ScalarE tanh (~230 ns/row). The per-partition-bias fusion path
    (~510 ns/row on ScalarE) lost to all-DVE staging once balance was judged
    by busy-interval UNION per engine rather than duration sums (instruction
    durations embed sem-waits and overstate DVE load ~30%).
  - Reduction over u on the TensorE: scale is replicated to a (128, 32)
    stationary operand (M=32) and each matmul streams two tanh rows
    (N=512, one PSUM bank); tile_position=(0, 32c) packs 4 such matvecs
    into disjoint column strips of the PE array, which both runs them
    concurrently and lands score rows on 4 different PSUM partitions
    (32c), so one full-width DVE copy evacuates 16 rows at once.
    PSUM bank rule honored: each matmul covers exactly one bank, so
    start=(j==0)/stop=(j==3) per slot.
  - Engines are partition-locked (no cross-partition moves), so score rows
    bounce through a DRAM staging buffer: 4 contiguous rows per strip-store,
    then one gather DMA rebuilds the (t, s) tile per half-batch.
  - softmax over the free dim without max-subtraction (|scores| <= 22 since
    |tanh|<=1 and sum|scale| ~ 22 -> exp stays comfortably in fp32 range);
    tails run per 64-row half-batch so they overlap the main loop.
  - context = attn @ value via PE transposes of attn + 2 matmuls.

Note: an all-masked row would produce NaN (reference's max-subtraction gives
uniform weights instead); the problem spec fixes mask = all-ones, and any
partially-masked row matches the reference exactly.
"""

import numpy as np
from contextlib import ExitStack

import concourse.bass as bass
from concourse import bacc
import concourse.tile as tile
from concourse import mybir
from concourse.bass import ts
from concourse.bass_utils import run_bass_kernel_spmd
from concourse.masks import make_identity

AF = mybir.ActivationFunctionType
F32 = mybir.dt.float32
F16 = mybir.dt.float16
U8 = mybir.dt.uint8

B, T, S, D, U = 16, 128, 256, 512, 512
NCORES = 8
BPC = B // NCORES  # batches per core
NJ = U // 128      # u-tiles
ND = D // 128      # d-tiles
NK = S // 128      # s-tiles
GA = 32            # t-rows per tanh group (2 PSUM sub-groups of 16)
DVE_ROWS = 32      # rows per group whose adds run on DVE (rest: ACT bias)
NGA = T // GA
NEG = -1e9


def build_bass():
    nc = bacc.Bacc("TRN2", target_bir_lowering=False, debug=False)

    query_d = nc.dram_tensor("query", [BPC, T, D], F32, kind="ExternalInput")
    value_d = nc.dram_tensor("value", [BPC, S, D], F32, kind="ExternalInput")
    mask_d = nc.dram_tensor("mask", [1, BPC, S], U8, kind="ExternalInput")
    w1_d = nc.dram_tensor("W1", [128, ND, U], F32, kind="ExternalInput")   # [p,i,u] = W1[i*128+p, u]
    w2_d = nc.dram_tensor("W2", [128, ND, U], F32, kind="ExternalInput")
    scale_d = nc.dram_tensor("scale", [128, NJ, 32], F16, kind="ExternalInput")  # [p,j,m] = scale[j*128+p]

    ctx_d = nc.dram_tensor("context", [BPC, T, D], F32, kind="ExternalOutput")
    attn_d = nc.dram_tensor("attn", [BPC, T, S], F32, kind="ExternalOutput")
    # Separate staging tensors per (batch, half) so each tail's gather DMA
    # depends only on its own 8 stores, not on every store of both batches
    # (coarse DRAM dep tracking otherwise serializes all tails to the end).
    stage_d = [[nc.dram_tensor(f"scores_stage_b{b}h{h}", [64 * S], F32)
                for h in range(2)] for b in range(BPC)]

    with tile.TileContext(nc) as tc, ExitStack() as ctx:
        consts = ctx.enter_context(tc.tile_pool(name="consts", bufs=1))
        pb = ctx.enter_context(tc.tile_pool(name="perbatch", bufs=2))
        stag_pool = ctx.enter_context(tc.tile_pool(name="stag", bufs=4))
        tanh_pool = ctx.enter_context(tc.tile_pool(name="tanh", bufs=4))
        row_pool = ctx.enter_context(tc.tile_pool(name="rowbuf", bufs=2))
        ps_small = ctx.enter_context(tc.tile_pool(name="ps_small", bufs=2, space="PSUM"))
        ps_rows = ctx.enter_context(tc.tile_pool(name="ps_rows", bufs=2, space="PSUM"))
        ps_ctx = ctx.enter_context(tc.tile_pool(name="ps_ctx", bufs=1, space="PSUM"))

        # ---- constants ----
        # Preload the tanh table set (~2.7us) while input DMAs run.
        warm = consts.tile([128, 1], F32)
        nc.vector.memset(warm[:], 0.0)
        nc.scalar.activation(warm[:], warm[:], AF.Tanh)

        ident = consts.tile([128, 128], F32)
        make_identity(nc, ident)

        scale16 = consts.tile([128, NJ, 32], F16)
        nc.sync.dma_start(scale16[:], scale_d[:])
        # gpsimd DMA casts fp32 DRAM -> fp16 SBUF directly (keeps the big
        # weight loads off the sync queue so query/value DMAs go first)
        w1_16 = consts.tile([128, ND, U], F16)
        nc.gpsimd.dma_start(w1_16[:], w1_d[:])
        w2_16 = consts.tile([128, ND, U], F16)
        nc.gpsimd.dma_start(w2_16[:], w2_d[:])

        qTs, kTs, v_nats, mb_bcs = [], [], [], []
        for b in range(BPC):
            # ---- load ----
            q_nat = pb.tile([128, D], F32)                      # (t, d)
            nc.sync.dma_start(q_nat[:], query_d[b])
            v_nat = pb.tile([128, NK, D], F32)                  # (s%128, k, d)
            nc.sync.dma_start(v_nat[:], value_d[b].rearrange("(k p) d -> p k d", p=128))

            # mask bias broadcast to (128, S):  (mask-1)*1e9
            mb_u8 = pb.tile([128, S], U8)
            mask_bc = bass.AP(
                tensor=mask_d.ap().tensor, offset=b * S,
                ap=[[0, 128], [1, S]],
            )
            nc.sync.dma_start(mb_u8[:], mask_bc)
            mb_bc = pb.tile([128, S], F32)
            nc.vector.tensor_scalar(
                mb_bc[:], mb_u8[:], 1e9, NEG,
                mybir.AluOpType.mult, mybir.AluOpType.add,
            )

            # ---- transpose query -> qTin (d on partitions) ----
            qTin = pb.tile([128, ND, 128], F16)                 # (d%128, i, t)
            for i in range(ND):
                ps_t = ps_small.tile([128, 128], F32, tag="ps_prep")
                nc.tensor.transpose(ps_t[:], q_nat[:, ts(i, 128)], ident[:])
                nc.vector.tensor_copy(qTin[:, i, :], ps_t[:])

            # ---- transpose value -> vT (d on partitions) ----
            vT = pb.tile([128, ND, S], F16)                     # (d%128, i, s)
            for i in range(ND):
                for k in range(NK):
                    ps_t = ps_small.tile([128, 128], F32, tag="ps_prep")
                    nc.tensor.transpose(ps_t[:], v_nat[:, k, ts(i, 128)], ident[:])
                    nc.vector.tensor_copy(vT[:, i, ts(k, 128)], ps_t[:])

            # ---- qT[u_j, t] = sum_i W1[d_i, u_j].T @ qTin[d_i, t] ----
            qT = pb.tile([128, NJ, 128], F32)   # fp32: DVE scalar operand
            for j in range(NJ):
                ps_q = ps_small.tile([128, 128], F32, tag="ps_prep")
                for i in range(ND):
                    nc.tensor.matmul(
                        ps_q[:], w1_16[:, i, ts(j, 128)], qTin[:, i, :],
                        start=(i == 0), stop=(i == ND - 1),
                    )
                nc.vector.tensor_copy(qT[:, j, :], ps_q[:])

            # ---- kT[u_j, s] = sum_i W2[d_i, u_j].T @ vT[d_i, s] ----
            kT = pb.tile([128, NJ, S], F16)                     # (u%128, j, s)
            for j in range(NJ):
                ps_k = ps_small.tile([128, S], F32, tag="ps_prep")
                for i in range(ND):
                    nc.tensor.matmul(
                        ps_k[:], w2_16[:, i, ts(j, 128)], vT[:, i, :],
                        start=(i == 0), stop=(i == ND - 1),
                    )
                nc.vector.tensor_copy(kT[:, j, :], ps_k[:])
            qTs.append(qT); kTs.append(kT); v_nats.append(v_nat); mb_bcs.append(mb_bc)

        # ---- main loops, batches interleaved per row-group ----
        # Per GA=16-row group: DVE builds tanh args for the first DVE_ROWS[j]
        # rows via tensor_scalar broadcast-add; ScalarE handles the remaining
        # rows fused into its tanh via the per-partition bias operand.
        # Row pair p (rows 2p, 2p+1) -> PE col-strip c=p//2 (tile_position
        # (0,32c), M=32 replicated scale so a whole strip fills), PSUM half
        # h=p%2. Strip c holds rows [4c, 4c+4): one wide DVE copy evacuates
        # 16 rows, one DMA per strip stores 4 contiguous rows to a DRAM
        # staging buffer (engines cannot scatter across partitions).
        for ga in range(NGA):
            for b in range(BPC):
                qT, kT = qTs[b], kTs[b]
                tanh_tiles = []
                for j in range(NJ):
                    stag = stag_pool.tile([128, DVE_ROWS * S], F16)
                    for r in range(DVE_ROWS):
                        t = ga * GA + r
                        nc.vector.tensor_scalar_add(
                            stag[:, ts(r, S)], kT[:, j, :], qT[:, j, t:t + 1],
                        )
                    tanh_t = tanh_pool.tile([128, GA * S], F16)
                    nc.scalar.activation(
                        tanh_t[:, 0:DVE_ROWS * S], stag[:], AF.Tanh)
                    for r in range(DVE_ROWS, GA):
                        t = ga * GA + r
                        nc.scalar.activation(
                            tanh_t[:, ts(r, S)], kT[:, j, :], AF.Tanh,
                            bias=qT[:, j, t:t + 1],
                        )
                    tanh_tiles.append(tanh_t)
                for sub in range(GA // 16):
                    prow = ps_rows.tile([128, 4 * S], F32)
                    for j in range(NJ):
                        for p in range(8):
                            c, h = p // 2, p % 2
                            r = sub * 16 + 2 * p
                            nc.tensor.matmul(
                                prow[32 * c:32 * c + 32, ts(h, 2 * S)],
                                scale16[:, j, :], tanh_tiles[j][:, r * S:(r + 2) * S],
                                start=(j == 0), stop=(j == NJ - 1),
                                tile_position=(0, 32 * c),
                                skip_group_check=True,
                            )
                    rowbuf = row_pool.tile([128, 4 * S], F32)
                    nc.vector.tensor_copy(rowbuf[:], prow[:])
                    for c in range(4):
                        half = ga // 2
                        base = ((ga % 2) * 32 + sub * 16 + 4 * c) * S
                        nc.sync.dma_start(
                            stage_d[b][half][base:base + 4 * S].rearrange("(o x) -> o x", o=1),
                            rowbuf[32 * c:32 * c + 1, :],
                        )

        # ---- tails: softmax + context, in half-batches so they overlap ----
        attnTs = {}
        ps_cs = {}
        for b in range(BPC):
            attnTs[b] = pb.tile([128, NK, 128], F32, name=f"attnT{b}", tag=f"attnT{b}")  # (s%128, k, t)
            ps_cs[b] = ps_ctx.tile([128, D], F32, name=f"ps_c{b}", tag=f"ps_c{b}")
        for b in range(BPC):
            for half in range(2):
                t0 = half * 64
                # gather staged scores rows [t0, t0+64)
                sc_h = pb.tile([64, S], F32, tag="sc_h")
                nc.sync.dma_start(
                    sc_h[:], stage_d[b][half].ap().rearrange("(t s) -> t s", s=S))
                masked = pb.tile([64, S], F32, tag="masked_h")
                nc.vector.tensor_add(masked[:], sc_h[:], mb_bcs[b][0:64, :])
                attn_e = pb.tile([64, S], F32, tag="attn_e_h")
                nc.scalar.activation(attn_e[:], masked[:], AF.Exp)
                ssum = pb.tile([64, 1], F32, tag="ssum_h")
                nc.vector.tensor_reduce(ssum[:], attn_e[:], axis=mybir.AxisListType.X,
                                        op=mybir.AluOpType.add)
                rsum = pb.tile([64, 1], F32, tag="rsum_h")
                nc.vector.reciprocal(rsum[:], ssum[:])
                attn_o = pb.tile([64, S], F32, tag="attn_o_h")
                nc.vector.tensor_scalar_mul(attn_o[:], attn_e[:], rsum[:])
                nc.sync.dma_start(attn_d[b, t0:t0 + 64, :], attn_o[:])

                # transpose this half into the batch attnT tile
                for k in range(NK):
                    ps_t = ps_small.tile([128, 64], F32, tag="ps_prep")
                    nc.tensor.transpose(ps_t[:], attn_o[:, ts(k, 128)], ident[0:64, 0:64])
                    nc.scalar.copy(attnTs[b][:, k, t0:t0 + 64], ps_t[:])
                # context rows [t0, t0+64): lhsT M=64 -> psum partition base t0
                for k in range(NK):
                    nc.tensor.matmul(
                        ps_cs[b][t0:t0 + 64, :], attnTs[b][:, k, t0:t0 + 64],
                        v_nats[b][:, k, :],
                        start=(k == 0), stop=(k == NK - 1),
                        skip_group_check=True,
                    )
            ctx_sb = pb.tile([128, D], F32)
            nc.scalar.copy(ctx_sb[:], ps_cs[b][:])
            nc.sync.dma_start(ctx_d[b], ctx_sb[:])

    nc.compile()
    return nc


_NC_CACHE = None


def _get_nc():
    global _NC_CACHE
    if _NC_CACHE is None:
        _NC_CACHE = build_bass()
    return _NC_CACHE


def _shard_inputs(query, value, mask, W1, W2, scale):
    w1_r = np.ascontiguousarray(
        np.asarray(W1, dtype=np.float32).reshape(ND, 128, U).transpose(1, 0, 2))
    w2_r = np.ascontiguousarray(
        np.asarray(W2, dtype=np.float32).reshape(ND, 128, U).transpose(1, 0, 2))
    scale_r = np.ascontiguousarray(np.broadcast_to(
        np.asarray(scale, dtype=np.float32).reshape(NJ, 128).T.astype(np.float16)[:, :, None],
        (128, NJ, 32)))
    in_maps = []
    for c in range(NCORES):
        sl = slice(c * BPC, (c + 1) * BPC)
        in_maps.append({
            "query": np.ascontiguousarray(np.asarray(query[sl], dtype=np.float32)),
            "value": np.ascontiguousarray(np.asarray(value[sl], dtype=np.float32)),
            "mask": np.ascontiguousarray(
                np.asarray(mask[sl]).astype(np.uint8).reshape(1, BPC, S)),
            "W1": w1_r,
            "W2": w2_r,
            "scale": scale_r,
        })
    return in_maps


def run(query, value, mask, W1, W2, scale, **run_kwargs):
    nc = _get_nc()
    in_maps = _shard_inputs(query, value, mask, W1, W2, scale)
    res = run_bass_kernel_spmd(nc, in_maps, core_ids=list(range(NCORES)), **run_kwargs)
    context = np.concatenate([r["context"] for r in res.results], axis=0)
    attn = np.concatenate([r["attn"] for r in res.results], axis=0)
    return (context, attn), res


def kernel(query, value, mask, W1, W2, scale):
    (context, attn), _ = run(query, value, mask, W1, W2, scale)
    return context, attn


# revision 32
# speedup vs baseline: 1.0205x; 1.0124x over previous
"""Bahdanau (additive) attention kernel for Trainium2, 8 NeuronCores.

Math (per batch b):
    q = query @ W1                        (t, u)
    k = value @ W2                        (s, u)
    scores[t, s] = sum_u scale_u * tanh(q[t, u] + k[s, u])
    scores = where(mask[s], scores, -1e9)
    attn = softmax_s(scores)
    context = attn @ value

Sharding: data-parallel over batch — 16 batches, 2 per core, W1/W2/scale
replicated. Each core runs an identical Bass program (SPMD).

Per-core device algorithm (measured ~289 us/core, engine-balance limited:
ScalarE tanh stream vs VectorE broadcast-adds — the 268M
elementwise tanh stream is the fundamental cost and tanh exists only on
ScalarE at 1 elem/lane/cycle):
  - Prep: qT (u, t) fp32 and kT (u, s) fp16 built with PE transposes +
    fp16 matmuls so the contraction dim `u` lies on partitions. fp16 is used
    on every matmul path (fp32 matmuls lower to 2x LOW_HIGH passes); fp16
    keeps attn rel err ~5e-4.
  - Main loop over (32-row t-group, u-tile j), batches interleaved per group
    so batch boundaries pipeline: the tanh argument arg[u_j, s] =
    kT[u_j, s] + qT[u_j, t] is a broadcast-add; all 32 rows/group go through DVE
    tensor_scalar (per-partition scalar = qT column, ~280 ns/row — DVE
    per-instruction floor) into an fp16 staging tile consumed by one big
    ScalarE tanh (~230 ns/row); the other 5 rows/group fuse the add into
    ScalarE tanh (~230 ns/row). The per-partition-bias fusion path
    (~510 ns/row on ScalarE) lost to all-DVE staging once balance was judged
    by busy-interval UNION per engine rather than duration sums (instruction
    durations embed sem-waits and overstate DVE load ~30%).
  - Reduction over u on the TensorE: scale is replicated to a (128, 32)
    stationary operand (M=32) and each matmul streams two tanh rows
    (N=512, one PSUM bank); tile_position=(0, 32c) packs 4 such matvecs
    into disjoint column strips of the PE array, which both runs them
    concurrently and lands score rows on 4 different PSUM partitions
    (32c), so one full-width DVE copy evacuates 16 rows at once.
    PSUM bank rule honored: each matmul covers exactly one bank, so
    start=(j==0)/stop=(j==3) per slot.
  - Engines are partition-locked (no cross-partition moves), so score rows
    bounce through a DRAM staging buffer: 4 contiguous rows per strip-store,
    then one gather DMA rebuilds the (t, s) tile per half-batch.
  - softmax over the free dim without max-subtraction (|scores| <= 22 since
    |tanh|<=1 and sum|scale| ~ 22 -> exp stays comfortably in fp32 range);
    tails run per 64-row half-batch so they overlap the main loop.
  - context = attn @ value via PE transposes of attn + 2 matmuls.

Note: an all-masked row would produce NaN (reference's max-subtraction gives
uniform weights instead); the problem spec fixes mask = all-ones, and any
partially-masked row matches the reference exactly.
"""

import numpy as np
from contextlib import ExitStack

import concourse.bass as bass
from concourse import bacc
import concourse.tile as tile
from concourse import mybir
from concourse.bass import ts
from concourse.bass_utils import run_bass_kernel_spmd
from concourse.masks import make_identity

AF = mybir.ActivationFunctionType
F32 = mybir.dt.float32
F16 = mybir.dt.float16
U8 = mybir.dt.uint8

B, T, S, D, U = 16, 128, 256, 512, 512
NCORES = 8
BPC = B // NCORES  # batches per core
NJ = U // 128      # u-tiles
ND = D // 128      # d-tiles
NK = S // 128      # s-tiles
GA = 32            # t-rows per tanh group (2 PSUM sub-groups of 16)
DVE_ROWS = 32      # rows per group whose adds run on DVE (rest: ACT bias)
NGA = T // GA
NEG = -1e9


def build_bass():
    nc = bacc.Bacc("TRN2", target_bir_lowering=False, debug=False)

    query_d = nc.dram_tensor("query", [BPC, T, D], F32, kind="ExternalInput")
    value_d = nc.dram_tensor("value", [BPC, S, D], F32, kind="ExternalInput")
    mask_d = nc.dram_tensor("mask", [1, BPC, S], U8, kind="ExternalInput")
    w1_d = nc.dram_tensor("W1", [128, ND, U], F32, kind="ExternalInput")   # [p,i,u] = W1[i*128+p, u]
    w2_d = nc.dram_tensor("W2", [128, ND, U], F32, kind="ExternalInput")
    scale_d = nc.dram_tensor("scale", [128, NJ, 32], F16, kind="ExternalInput")  # [p,j,m] = scale[j*128+p]

    ctx_d = nc.dram_tensor("context", [BPC, T, D], F32, kind="ExternalOutput")
    attn_d = nc.dram_tensor("attn", [BPC, T, S], F32, kind="ExternalOutput")
    # Separate staging tensors per (batch, half) so each tail's gather DMA
    # depends only on its own 8 stores, not on every store of both batches
    # (coarse DRAM dep tracking otherwise serializes all tails to the end).
    stage_d = [[nc.dram_tensor(f"scores_stage_b{b}h{h}", [64 * S], F32)
                for h in range(2)] for b in range(BPC)]

    with tile.TileContext(nc) as tc, ExitStack() as ctx:
        consts = ctx.enter_context(tc.tile_pool(name="consts", bufs=1))
        pb = ctx.enter_context(tc.tile_pool(name="perbatch", bufs=2))
        stag_pool = ctx.enter_context(tc.tile_pool(name="stag", bufs=4))
        tanh_pool = ctx.enter_context(tc.tile_pool(name="tanh", bufs=4))
        row_pool = ctx.enter_context(tc.tile_pool(name="rowbuf", bufs=2))
        ps_small = ctx.enter_context(tc.tile_pool(name="ps_small", bufs=2, space="PSUM"))
        ps_rows = ctx.enter_context(tc.tile_pool(name="ps_rows", bufs=2, space="PSUM"))
        ps_ctx = ctx.enter_context(tc.tile_pool(name="ps_ctx", bufs=1, space="PSUM"))

        # ---- constants ----
        # Preload the tanh table set (~2.7us) while input DMAs run.
        warm = consts.tile([128, 1], F32)
        nc.vector.memset(warm[:], 0.0)
        nc.scalar.activation(warm[:], warm[:], AF.Tanh)

        ident = consts.tile([128, 128], F32)
        make_identity(nc, ident)

        scale16 = consts.tile([128, NJ, 32], F16)
        nc.sync.dma_start(scale16[:], scale_d[:])
        # gpsimd DMA casts fp32 DRAM -> fp16 SBUF directly (keeps the big
        # weight loads off the sync queue so query/value DMAs go first)
        w1_16 = consts.tile([128, ND, U], F16)
        nc.gpsimd.dma_start(w1_16[:], w1_d[:])
        w2_16 = consts.tile([128, ND, U], F16)
        nc.gpsimd.dma_start(w2_16[:], w2_d[:])

        qTs, kTs, v_nats, mb_bcs = [], [], [], []
        for b in range(BPC):
            # ---- load ----
            q_nat = pb.tile([128, D], F32)                      # (t, d)
            nc.sync.dma_start(q_nat[:], query_d[b])
            v_nat = pb.tile([128, NK, D], F32)                  # (s%128, k, d)
            nc.sync.dma_start(v_nat[:], value_d[b].rearrange("(k p) d -> p k d", p=128))

            # mask bias broadcast to (128, S):  (mask-1)*1e9
            mb_u8 = pb.tile([128, S], U8)
            mask_bc = bass.AP(
                tensor=mask_d.ap().tensor, offset=b * S,
                ap=[[0, 128], [1, S]],
            )
            nc.sync.dma_start(mb_u8[:], mask_bc)
            mb_bc = pb.tile([128, S], F32)
            nc.vector.tensor_scalar(
                mb_bc[:], mb_u8[:], 1e9, NEG,
                mybir.AluOpType.mult, mybir.AluOpType.add,
            )

            # ---- transpose query -> qTin (d on partitions) ----
            qTin = pb.tile([128, ND, 128], F16)                 # (d%128, i, t)
            for i in range(ND):
                ps_t = ps_small.tile([128, 128], F32, tag="ps_prep")
                nc.tensor.transpose(ps_t[:], q_nat[:, ts(i, 128)], ident[:])
                nc.vector.tensor_copy(qTin[:, i, :], ps_t[:])

            # ---- transpose value -> vT (d on partitions) ----
            vT = pb.tile([128, ND, S], F16)                     # (d%128, i, s)
            for i in range(ND):
                for k in range(NK):
                    ps_t = ps_small.tile([128, 128], F32, tag="ps_prep")
                    nc.tensor.transpose(ps_t[:], v_nat[:, k, ts(i, 128)], ident[:])
                    nc.vector.tensor_copy(vT[:, i, ts(k, 128)], ps_t[:])

            # ---- qT[u_j, t] = sum_i W1[d_i, u_j].T @ qTin[d_i, t] ----
            # all 4 j-blocks share one PSUM bank (512 fp32); start=True only
            # clears has_written bits, so earlier blocks' data survives
            qT = pb.tile([128, NJ, 128], F32)   # fp32: DVE scalar operand
            ps_q = ps_small.tile([128, 512], F32, tag="ps_prep")
            for j in range(NJ):
                for i in range(ND):
                    nc.tensor.matmul(
                        ps_q[:, ts(j, 128)], w1_16[:, i, ts(j, 128)], qTin[:, i, :],
                        start=(i == 0), stop=(i == ND - 1),
                        skip_group_check=True,
                    )
            nc.vector.tensor_copy(qT[:], ps_q[:].rearrange("p (j t) -> p j t", j=NJ))

            # ---- kT[u_j, s] = sum_i W2[d_i, u_j].T @ vT[d_i, s] ----
            # j-pairs share one PSUM bank (2 x 256 fp32) -> 2 wide copies
            kT = pb.tile([128, NJ, S], F16)                     # (u%128, j, s)
            for jp in range(NJ // 2):
                ps_k = ps_small.tile([128, 2 * S], F32, tag="ps_prep")
                for j2 in range(2):
                    j = jp * 2 + j2
                    for i in range(ND):
                        nc.tensor.matmul(
                            ps_k[:, ts(j2, S)], w2_16[:, i, ts(j, 128)], vT[:, i, :],
                            start=(i == 0), stop=(i == ND - 1),
                            skip_group_check=True,
                        )
                nc.vector.tensor_copy(
                    kT[:, jp * 2:jp * 2 + 2, :],
                    ps_k[:].rearrange("p (j s) -> p j s", j=2))
            qTs.append(qT); kTs.append(kT); v_nats.append(v_nat); mb_bcs.append(mb_bc)

        # ---- main loops, batches interleaved per row-group ----
        # Per GA=16-row group: DVE builds tanh args for the first DVE_ROWS[j]
        # rows via tensor_scalar broadcast-add; ScalarE handles the remaining
        # rows fused into its tanh via the per-partition bias operand.
        # Row pair p (rows 2p, 2p+1) -> PE col-strip c=p//2 (tile_position
        # (0,32c), M=32 replicated scale so a whole strip fills), PSUM half
        # h=p%2. Strip c holds rows [4c, 4c+4): one wide DVE copy evacuates
        # 16 rows, one DMA per strip stores 4 contiguous rows to a DRAM
        # staging buffer (engines cannot scatter across partitions).
        for ga in range(NGA):
            for b in range(BPC):
                qT, kT = qTs[b], kTs[b]
                tanh_tiles = []
                for j in range(NJ):
                    stag = stag_pool.tile([128, DVE_ROWS * S], F16)
                    for r in range(DVE_ROWS):
                        t = ga * GA + r
                        nc.vector.tensor_scalar_add(
                            stag[:, ts(r, S)], kT[:, j, :], qT[:, j, t:t + 1],
                        )
                    tanh_t = tanh_pool.tile([128, GA * S], F16)
                    nc.scalar.activation(
                        tanh_t[:, 0:DVE_ROWS * S], stag[:], AF.Tanh)
                    for r in range(DVE_ROWS, GA):
                        t = ga * GA + r
                        nc.scalar.activation(
                            tanh_t[:, ts(r, S)], kT[:, j, :], AF.Tanh,
                            bias=qT[:, j, t:t + 1],
                        )
                    tanh_tiles.append(tanh_t)
                for sub in range(GA // 16):
                    prow = ps_rows.tile([128, 4 * S], F32)
                    for j in range(NJ):
                        for p in range(8):
                            c, h = p // 2, p % 2
                            r = sub * 16 + 2 * p
                            nc.tensor.matmul(
                                prow[32 * c:32 * c + 32, ts(h, 2 * S)],
                                scale16[:, j, :], tanh_tiles[j][:, r * S:(r + 2) * S],
                                start=(j == 0), stop=(j == NJ - 1),
                                tile_position=(0, 32 * c),
                                skip_group_check=True,
                            )
                    rowbuf = row_pool.tile([128, 4 * S], F32)
                    nc.vector.tensor_copy(rowbuf[:], prow[:])
                    for c in range(4):
                        half = ga // 2
                        base = ((ga % 2) * 32 + sub * 16 + 4 * c) * S
                        nc.sync.dma_start(
                            stage_d[b][half][base:base + 4 * S].rearrange("(o x) -> o x", o=1),
                            rowbuf[32 * c:32 * c + 1, :],
                        )

        # ---- tails: softmax + context, in half-batches so they overlap ----
        attnTs = {}
        ps_cs = {}
        for b in range(BPC):
            attnTs[b] = pb.tile([128, NK, 128], F32, name=f"attnT{b}", tag=f"attnT{b}")  # (s%128, k, t)
            ps_cs[b] = ps_ctx.tile([128, D], F32, name=f"ps_c{b}", tag=f"ps_c{b}")
        for b in range(BPC):
            for half in range(2):
                t0 = half * 64
                # gather staged scores rows [t0, t0+64)
                sc_h = pb.tile([64, S], F32, tag="sc_h")
                nc.sync.dma_start(
                    sc_h[:], stage_d[b][half].ap().rearrange("(t s) -> t s", s=S))
                masked = pb.tile([64, S], F32, tag="masked_h")
                nc.vector.tensor_add(masked[:], sc_h[:], mb_bcs[b][0:64, :])
                attn_e = pb.tile([64, S], F32, tag="attn_e_h")
                nc.scalar.activation(attn_e[:], masked[:], AF.Exp)
                ssum = pb.tile([64, 1], F32, tag="ssum_h")
                nc.vector.tensor_reduce(ssum[:], attn_e[:], axis=mybir.AxisListType.X,
                                        op=mybir.AluOpType.add)
                rsum = pb.tile([64, 1], F32, tag="rsum_h")
                nc.vector.reciprocal(rsum[:], ssum[:])
                attn_o = pb.tile([64, S], F32, tag="attn_o_h")
                nc.vector.tensor_scalar_mul(attn_o[:], attn_e[:], rsum[:])
                nc.sync.dma_start(attn_d[b, t0:t0 + 64, :], attn_o[:])

                # transpose this half into the batch attnT tile
                for k in range(NK):
                    ps_t = ps_small.tile([128, 64], F32, tag="ps_prep")
                    nc.tensor.transpose(ps_t[:], attn_o[:, ts(k, 128)], ident[0:64, 0:64])
                    nc.scalar.copy(attnTs[b][:, k, t0:t0 + 64], ps_t[:])
                # context rows [t0, t0+64): lhsT M=64 -> psum partition base t0
                for k in range(NK):
                    nc.tensor.matmul(
                        ps_cs[b][t0:t0 + 64, :], attnTs[b][:, k, t0:t0 + 64],
                        v_nats[b][:, k, :],
                        start=(k == 0), stop=(k == NK - 1),
                        skip_group_check=True,
                    )
            ctx_sb = pb.tile([128, D], F32)
            nc.scalar.copy(ctx_sb[:], ps_cs[b][:])
            nc.sync.dma_start(ctx_d[b], ctx_sb[:])

    nc.compile()
    return nc


_NC_CACHE = None


def _get_nc():
    global _NC_CACHE
    if _NC_CACHE is None:
        _NC_CACHE = build_bass()
    return _NC_CACHE


def _shard_inputs(query, value, mask, W1, W2, scale):
    w1_r = np.ascontiguousarray(
        np.asarray(W1, dtype=np.float32).reshape(ND, 128, U).transpose(1, 0, 2))
    w2_r = np.ascontiguousarray(
        np.asarray(W2, dtype=np.float32).reshape(ND, 128, U).transpose(1, 0, 2))
    scale_r = np.ascontiguousarray(np.broadcast_to(
        np.asarray(scale, dtype=np.float32).reshape(NJ, 128).T.astype(np.float16)[:, :, None],
        (128, NJ, 32)))
    in_maps = []
    for c in range(NCORES):
        sl = slice(c * BPC, (c + 1) * BPC)
        in_maps.append({
            "query": np.ascontiguousarray(np.asarray(query[sl], dtype=np.float32)),
            "value": np.ascontiguousarray(np.asarray(value[sl], dtype=np.float32)),
            "mask": np.ascontiguousarray(
                np.asarray(mask[sl]).astype(np.uint8).reshape(1, BPC, S)),
            "W1": w1_r,
            "W2": w2_r,
            "scale": scale_r,
        })
    return in_maps


def run(query, value, mask, W1, W2, scale, **run_kwargs):
    nc = _get_nc()
    in_maps = _shard_inputs(query, value, mask, W1, W2, scale)
    res = run_bass_kernel_spmd(nc, in_maps, core_ids=list(range(NCORES)), **run_kwargs)
    context = np.concatenate([r["context"] for r in res.results], axis=0)
    attn = np.concatenate([r["attn"] for r in res.results], axis=0)
    return (context, attn), res


def kernel(query, value, mask, W1, W2, scale):
    (context, attn), _ = run(query, value, mask, W1, W2, scale)
    return context, attn


# revision 33
# speedup vs baseline: 1.0226x; 1.0020x over previous
"""Bahdanau (additive) attention kernel for Trainium2, 8 NeuronCores.

Math (per batch b):
    q = query @ W1                        (t, u)
    k = value @ W2                        (s, u)
    scores[t, s] = sum_u scale_u * tanh(q[t, u] + k[s, u])
    scores = where(mask[s], scores, -1e9)
    attn = softmax_s(scores)
    context = attn @ value

Sharding: data-parallel over batch — 16 batches, 2 per core, W1/W2/scale
replicated. Each core runs an identical Bass program (SPMD).

Per-core device algorithm (measured ~289 us/core, engine-balance limited:
ScalarE tanh stream vs VectorE broadcast-adds — the 268M
elementwise tanh stream is the fundamental cost and tanh exists only on
ScalarE at 1 elem/lane/cycle):
  - Prep: qT (u, t) fp32 and kT (u, s) fp16 built with PE transposes +
    fp16 matmuls so the contraction dim `u` lies on partitions. fp16 is used
    on every matmul path (fp32 matmuls lower to 2x LOW_HIGH passes); fp16
    keeps attn rel err ~5e-4.
  - Main loop over (32-row t-group, u-tile j), batches interleaved per group
    so batch boundaries pipeline: the tanh argument arg[u_j, s] =
    kT[u_j, s] + qT[u_j, t] is a broadcast-add; all 32 rows/group go through DVE
    tensor_scalar (per-partition scalar = qT column, ~280 ns/row — DVE
    per-instruction floor) into an fp16 staging tile consumed by one big
    ScalarE tanh (~230 ns/row); the other 5 rows/group fuse the add into
    ScalarE tanh (~230 ns/row). The per-partition-bias fusion path
    (~510 ns/row on ScalarE) lost to all-DVE staging once balance was judged
    by busy-interval UNION per engine rather than duration sums (instruction
    durations embed sem-waits and overstate DVE load ~30%).
  - Reduction over u on the TensorE: scale is replicated to a (128, 32)
    stationary operand (M=32) and each matmul streams two tanh rows
    (N=512, one PSUM bank); tile_position=(0, 32c) packs 4 such matvecs
    into disjoint column strips of the PE array, which both runs them
    concurrently and lands score rows on 4 different PSUM partitions
    (32c), so one full-width DVE copy evacuates 16 rows at once.
    PSUM bank rule honored: each matmul covers exactly one bank, so
    start=(j==0)/stop=(j==3) per slot.
  - Engines are partition-locked (no cross-partition moves), so score rows
    bounce through a DRAM staging buffer: 4 contiguous rows per strip-store,
    then one gather DMA rebuilds the (t, s) tile per half-batch.
  - softmax over the free dim without max-subtraction (|scores| <= 22 since
    |tanh|<=1 and sum|scale| ~ 22 -> exp stays comfortably in fp32 range);
    tails run per 64-row half-batch so they overlap the main loop.
  - context = attn @ value via PE transposes of attn + 2 matmuls.

Note: an all-masked row would produce NaN (reference's max-subtraction gives
uniform weights instead); the problem spec fixes mask = all-ones, and any
partially-masked row matches the reference exactly.
"""

import numpy as np
from contextlib import ExitStack

import concourse.bass as bass
from concourse import bacc
import concourse.tile as tile
from concourse import mybir
from concourse.bass import ts
from concourse.bass_utils import run_bass_kernel_spmd
from concourse.masks import make_identity

AF = mybir.ActivationFunctionType
F32 = mybir.dt.float32
F16 = mybir.dt.float16
U8 = mybir.dt.uint8

B, T, S, D, U = 16, 128, 256, 512, 512
NCORES = 8
BPC = B // NCORES  # batches per core
NJ = U // 128      # u-tiles
ND = D // 128      # d-tiles
NK = S // 128      # s-tiles
GA = 32            # t-rows per tanh group (2 PSUM sub-groups of 16)
DVE_ROWS = 32      # rows per group whose adds run on DVE (rest: ACT bias)
NGA = T // GA
NEG = -1e9


def build_bass():
    nc = bacc.Bacc("TRN2", target_bir_lowering=False, debug=False)

    query_d = nc.dram_tensor("query", [BPC, T, D], F32, kind="ExternalInput")
    value_d = nc.dram_tensor("value", [BPC, S, D], F32, kind="ExternalInput")
    mask_d = nc.dram_tensor("mask", [1, BPC, S], U8, kind="ExternalInput")
    w1_d = nc.dram_tensor("W1", [128, ND, U], F32, kind="ExternalInput")   # [p,i,u] = W1[i*128+p, u]
    w2_d = nc.dram_tensor("W2", [128, ND, U], F32, kind="ExternalInput")
    scale_d = nc.dram_tensor("scale", [128, NJ, 32], F16, kind="ExternalInput")  # [p,j,m] = scale[j*128+p]

    ctx_d = nc.dram_tensor("context", [BPC, T, D], F32, kind="ExternalOutput")
    attn_d = nc.dram_tensor("attn", [BPC, T, S], F32, kind="ExternalOutput")
    # Separate staging tensors per (batch, half) so each tail's gather DMA
    # depends only on its own 8 stores, not on every store of both batches
    # (coarse DRAM dep tracking otherwise serializes all tails to the end).
    stage_d = [[nc.dram_tensor(f"scores_stage_b{b}h{h}", [64 * S], F32)
                for h in range(2)] for b in range(BPC)]

    with tile.TileContext(nc) as tc, ExitStack() as ctx:
        consts = ctx.enter_context(tc.tile_pool(name="consts", bufs=1))
        pb = ctx.enter_context(tc.tile_pool(name="perbatch", bufs=2))
        stag_pool = ctx.enter_context(tc.tile_pool(name="stag", bufs=4))
        tanh_pool = ctx.enter_context(tc.tile_pool(name="tanh", bufs=4))
        row_pool = ctx.enter_context(tc.tile_pool(name="rowbuf", bufs=2))
        ps_small = ctx.enter_context(tc.tile_pool(name="ps_small", bufs=2, space="PSUM"))
        ps_rows = ctx.enter_context(tc.tile_pool(name="ps_rows", bufs=2, space="PSUM"))
        ps_ctx = ctx.enter_context(tc.tile_pool(name="ps_ctx", bufs=1, space="PSUM"))

        # ---- constants ----
        # Preload the tanh table set (~2.7us) while input DMAs run.
        warm = consts.tile([128, 1], F32)
        nc.vector.memset(warm[:], 0.0)
        nc.scalar.activation(warm[:], warm[:], AF.Tanh)

        ident = consts.tile([128, 128], F32)
        make_identity(nc, ident)

        scale16 = consts.tile([128, NJ, 32], F16)
        nc.sync.dma_start(scale16[:], scale_d[:])
        # gpsimd DMA casts fp32 DRAM -> fp16 SBUF directly (keeps the big
        # weight loads off the sync queue so query/value DMAs go first)
        w1_16 = consts.tile([128, ND, U], F16)
        nc.gpsimd.dma_start(w1_16[:], w1_d[:])
        w2_16 = consts.tile([128, ND, U], F16)
        nc.gpsimd.dma_start(w2_16[:], w2_d[:])

        qTs, kTs, v_nats, mb_bcs = [], [], [], []
        for b in range(BPC):
            # ---- load ----
            q_nat = pb.tile([128, D], F32)                      # (t, d)
            nc.sync.dma_start(q_nat[:], query_d[b])
            v_nat = pb.tile([128, NK, D], F32)                  # (s%128, k, d)
            nc.sync.dma_start(v_nat[:], value_d[b].rearrange("(k p) d -> p k d", p=128))

            # mask bias broadcast to (128, S):  (mask-1)*1e9
            mb_u8 = pb.tile([128, S], U8)
            mask_bc = bass.AP(
                tensor=mask_d.ap().tensor, offset=b * S,
                ap=[[0, 128], [1, S]],
            )
            nc.sync.dma_start(mb_u8[:], mask_bc)
            mb_bc = pb.tile([128, S], F32)
            nc.vector.tensor_scalar(
                mb_bc[:], mb_u8[:], 1e9, NEG,
                mybir.AluOpType.mult, mybir.AluOpType.add,
            )

            # ---- transpose query -> qTin (d on partitions) ----
            # 4 transposes share one PSUM bank; one wide evac copy
            qTin = pb.tile([128, ND, 128], F16)                 # (d%128, i, t)
            ps_t4 = ps_small.tile([128, 512], F32, tag="ps_prep")
            for i in range(ND):
                nc.tensor.transpose(ps_t4[:, ts(i, 128)], q_nat[:, ts(i, 128)], ident[:])
            nc.vector.tensor_copy(qTin[:], ps_t4[:].rearrange("p (i t) -> p i t", i=ND))

            # ---- transpose value -> vT (d on partitions) ----
            vT = pb.tile([128, ND, S], F16)                     # (d%128, i, s)
            for half in range(2):
                ps_t4 = ps_small.tile([128, 512], F32, tag="ps_prep")
                for n in range(4):
                    i, k = half * 2 + n // 2, n % 2
                    nc.tensor.transpose(ps_t4[:, ts(n, 128)], v_nat[:, k, ts(i, 128)], ident[:])
                nc.vector.tensor_copy(
                    vT[:, half * 2:half * 2 + 2, :],
                    ps_t4[:].rearrange("p (i s) -> p i s", i=2))

            # ---- qT[u_j, t] = sum_i W1[d_i, u_j].T @ qTin[d_i, t] ----
            # all 4 j-blocks share one PSUM bank (512 fp32); start=True only
            # clears has_written bits, so earlier blocks' data survives
            qT = pb.tile([128, NJ, 128], F32)   # fp32: DVE scalar operand
            ps_q = ps_small.tile([128, 512], F32, tag="ps_prep")
            for j in range(NJ):
                for i in range(ND):
                    nc.tensor.matmul(
                        ps_q[:, ts(j, 128)], w1_16[:, i, ts(j, 128)], qTin[:, i, :],
                        start=(i == 0), stop=(i == ND - 1),
                        skip_group_check=True,
                    )
            nc.vector.tensor_copy(qT[:], ps_q[:].rearrange("p (j t) -> p j t", j=NJ))

            # ---- kT[u_j, s] = sum_i W2[d_i, u_j].T @ vT[d_i, s] ----
            # j-pairs share one PSUM bank (2 x 256 fp32) -> 2 wide copies
            kT = pb.tile([128, NJ, S], F16)                     # (u%128, j, s)
            for jp in range(NJ // 2):
                ps_k = ps_small.tile([128, 2 * S], F32, tag="ps_prep")
                for j2 in range(2):
                    j = jp * 2 + j2
                    for i in range(ND):
                        nc.tensor.matmul(
                            ps_k[:, ts(j2, S)], w2_16[:, i, ts(j, 128)], vT[:, i, :],
                            start=(i == 0), stop=(i == ND - 1),
                            skip_group_check=True,
                        )
                nc.vector.tensor_copy(
                    kT[:, jp * 2:jp * 2 + 2, :],
                    ps_k[:].rearrange("p (j s) -> p j s", j=2))
            qTs.append(qT); kTs.append(kT); v_nats.append(v_nat); mb_bcs.append(mb_bc)

        # ---- main loops, batches interleaved per row-group ----
        # Per GA=16-row group: DVE builds tanh args for the first DVE_ROWS[j]
        # rows via tensor_scalar broadcast-add; ScalarE handles the remaining
        # rows fused into its tanh via the per-partition bias operand.
        # Row pair p (rows 2p, 2p+1) -> PE col-strip c=p//2 (tile_position
        # (0,32c), M=32 replicated scale so a whole strip fills), PSUM half
        # h=p%2. Strip c holds rows [4c, 4c+4): one wide DVE copy evacuates
        # 16 rows, one DMA per strip stores 4 contiguous rows to a DRAM
        # staging buffer (engines cannot scatter across partitions).
        for ga in range(NGA):
            for b in range(BPC):
                qT, kT = qTs[b], kTs[b]
                tanh_tiles = []
                for j in range(NJ):
                    stag = stag_pool.tile([128, DVE_ROWS * S], F16)
                    for r in range(DVE_ROWS):
                        t = ga * GA + r
                        nc.vector.tensor_scalar_add(
                            stag[:, ts(r, S)], kT[:, j, :], qT[:, j, t:t + 1],
                        )
                    tanh_t = tanh_pool.tile([128, GA * S], F16)
                    nc.scalar.activation(
                        tanh_t[:, 0:DVE_ROWS * S], stag[:], AF.Tanh)
                    for r in range(DVE_ROWS, GA):
                        t = ga * GA + r
                        nc.scalar.activation(
                            tanh_t[:, ts(r, S)], kT[:, j, :], AF.Tanh,
                            bias=qT[:, j, t:t + 1],
                        )
                    tanh_tiles.append(tanh_t)
                for sub in range(GA // 16):
                    prow = ps_rows.tile([128, 4 * S], F32)
                    for j in range(NJ):
                        for p in range(8):
                            c, h = p // 2, p % 2
                            r = sub * 16 + 2 * p
                            nc.tensor.matmul(
                                prow[32 * c:32 * c + 32, ts(h, 2 * S)],
                                scale16[:, j, :], tanh_tiles[j][:, r * S:(r + 2) * S],
                                start=(j == 0), stop=(j == NJ - 1),
                                tile_position=(0, 32 * c),
                                skip_group_check=True,
                            )
                    rowbuf = row_pool.tile([128, 4 * S], F32)
                    nc.vector.tensor_copy(rowbuf[:], prow[:])
                    for c in range(4):
                        half = ga // 2
                        base = ((ga % 2) * 32 + sub * 16 + 4 * c) * S
                        nc.sync.dma_start(
                            stage_d[b][half][base:base + 4 * S].rearrange("(o x) -> o x", o=1),
                            rowbuf[32 * c:32 * c + 1, :],
                        )

        # ---- tails: softmax + context, in half-batches so they overlap ----
        attnTs = {}
        ps_cs = {}
        for b in range(BPC):
            attnTs[b] = pb.tile([128, NK, 128], F32, name=f"attnT{b}", tag=f"attnT{b}")  # (s%128, k, t)
            ps_cs[b] = ps_ctx.tile([128, D], F32, name=f"ps_c{b}", tag=f"ps_c{b}")
        for b in range(BPC):
            for half in range(2):
                t0 = half * 64
                # gather staged scores rows [t0, t0+64)
                sc_h = pb.tile([64, S], F32, tag="sc_h")
                nc.sync.dma_start(
                    sc_h[:], stage_d[b][half].ap().rearrange("(t s) -> t s", s=S))
                masked = pb.tile([64, S], F32, tag="masked_h")
                nc.vector.tensor_add(masked[:], sc_h[:], mb_bcs[b][0:64, :])
                attn_e = pb.tile([64, S], F32, tag="attn_e_h")
                nc.scalar.activation(attn_e[:], masked[:], AF.Exp)
                ssum = pb.tile([64, 1], F32, tag="ssum_h")
                nc.vector.tensor_reduce(ssum[:], attn_e[:], axis=mybir.AxisListType.X,
                                        op=mybir.AluOpType.add)
                rsum = pb.tile([64, 1], F32, tag="rsum_h")
                nc.vector.reciprocal(rsum[:], ssum[:])
                attn_o = pb.tile([64, S], F32, tag="attn_o_h")
                nc.vector.tensor_scalar_mul(attn_o[:], attn_e[:], rsum[:])
                nc.sync.dma_start(attn_d[b, t0:t0 + 64, :], attn_o[:])

                # transpose this half into the batch attnT tile
                for k in range(NK):
                    ps_t = ps_small.tile([128, 64], F32, tag="ps_prep")
                    nc.tensor.transpose(ps_t[:], attn_o[:, ts(k, 128)], ident[0:64, 0:64])
                    nc.scalar.copy(attnTs[b][:, k, t0:t0 + 64], ps_t[:])
                # context rows [t0, t0+64): lhsT M=64 -> psum partition base t0
                for k in range(NK):
                    nc.tensor.matmul(
                        ps_cs[b][t0:t0 + 64, :], attnTs[b][:, k, t0:t0 + 64],
                        v_nats[b][:, k, :],
                        start=(k == 0), stop=(k == NK - 1),
                        skip_group_check=True,
                    )
            ctx_sb = pb.tile([128, D], F32)
            nc.scalar.copy(ctx_sb[:], ps_cs[b][:])
            nc.sync.dma_start(ctx_d[b], ctx_sb[:])

    nc.compile()
    return nc


_NC_CACHE = None


def _get_nc():
    global _NC_CACHE
    if _NC_CACHE is None:
        _NC_CACHE = build_bass()
    return _NC_CACHE


def _shard_inputs(query, value, mask, W1, W2, scale):
    w1_r = np.ascontiguousarray(
        np.asarray(W1, dtype=np.float32).reshape(ND, 128, U).transpose(1, 0, 2))
    w2_r = np.ascontiguousarray(
        np.asarray(W2, dtype=np.float32).reshape(ND, 128, U).transpose(1, 0, 2))
    scale_r = np.ascontiguousarray(np.broadcast_to(
        np.asarray(scale, dtype=np.float32).reshape(NJ, 128).T.astype(np.float16)[:, :, None],
        (128, NJ, 32)))
    in_maps = []
    for c in range(NCORES):
        sl = slice(c * BPC, (c + 1) * BPC)
        in_maps.append({
            "query": np.ascontiguousarray(np.asarray(query[sl], dtype=np.float32)),
            "value": np.ascontiguousarray(np.asarray(value[sl], dtype=np.float32)),
            "mask": np.ascontiguousarray(
                np.asarray(mask[sl]).astype(np.uint8).reshape(1, BPC, S)),
            "W1": w1_r,
            "W2": w2_r,
            "scale": scale_r,
        })
    return in_maps


def run(query, value, mask, W1, W2, scale, **run_kwargs):
    nc = _get_nc()
    in_maps = _shard_inputs(query, value, mask, W1, W2, scale)
    res = run_bass_kernel_spmd(nc, in_maps, core_ids=list(range(NCORES)), **run_kwargs)
    context = np.concatenate([r["context"] for r in res.results], axis=0)
    attn = np.concatenate([r["attn"] for r in res.results], axis=0)
    return (context, attn), res


def kernel(query, value, mask, W1, W2, scale):
    (context, attn), _ = run(query, value, mask, W1, W2, scale)
    return context, attn
